# revision 22
# baseline (speedup 1.0000x reference)
# Trainium2 Bass kernel for nn_ClsContrastLoss.
#
# Reference computation (bs=1024, d=1024, neg_num=32):
#   loss = -mean(log_softmax([cos(q,p), cos(q,n_1..32)] / T)[:, 0]) * CLS_W
#
# Sharding: data-parallel over bs across 8 cores. Each core handles 128
# samples (one per SBUF partition) and computes per-sample raw reductions:
#   s_pos = q.p, s_neg[j] = q.n_j   (DVE scalar_tensor_tensor with accum_out
#                                    -> fused multiply+reduce, one pass)
#   qq, pp, nn[j] = sum of squares  (ACT Square with accum_out)
# The tiny [1024, 67] -> scalar epilogue (sqrt/div/log-softmax/mean) runs on
# host in float64. Engine budgets per core (HW-measured): DMA 17.25 MB at
# ~417 GB/s = 41 us, DVE 33 x 1.26 us = 42 us, ACT 34 x 1.14 us = 39 us.
import numpy as np

N_CORES = 8
BS = 1024
D = 1024
NEG = 32
BS_LOC = BS // N_CORES  # 128 samples per core = one per partition
# negatives per DMA chunk: 1 MiB transfers pipeline well on HW and keep the
# per-chunk compute tail small
CHUNKS = [2] * 16
assert sum(CHUNKS) == NEG

TEMPERATURE = 0.05
CLS_W = 0.2
EPS = 1e-8

_CACHE = {}
LAST_RESULT = None  # BassKernelResults of the most recent run (for profiling)


def _build(chunks=None, qp_engine="sync", bufs=6, merged_out=True,
           alternate_rings=False, gp_every=0, bench_iters=0):
    import concourse.bacc as bacc
    import concourse.mybir as mybir
    import concourse.tile as tile

    if chunks is None:
        chunks = CHUNKS
    assert sum(chunks) == NEG

    f32 = mybir.dt.float32
    SQUARE = mybir.ActivationFunctionType.Square

    nc = bacc.Bacc("TRN2")
    q_ext = nc.dram_tensor("q", [BS_LOC, D], f32, kind="ExternalInput")
    p_ext = nc.dram_tensor("p", [BS_LOC, D], f32, kind="ExternalInput")
    # negatives reshaped host-side to [128, 32*1024]: row s = the 32 negatives
    # of sample s, concatenated (DRAM layout identical to [128*32, 1024])
    n_ext = nc.dram_tensor("n", [BS_LOC, NEG * D], f32, kind="ExternalInput")
    if merged_out:
        stats_out = nc.dram_tensor(
            "stats", [BS_LOC, 3 + 2 * NEG], f32, kind="ExternalOutput"
        )
    else:
        dve_out = nc.dram_tensor(
            "dve_stats", [BS_LOC, 1 + NEG], f32, kind="ExternalOutput"
        )
        act_out = nc.dram_tensor(
            "act_stats", [BS_LOC, 2 + NEG], f32, kind="ExternalOutput"
        )

    with tile.TileContext(nc) as tc:
        with (
            tc.tile_pool(name="io", bufs=1) as io,
            tc.tile_pool(name="negs", bufs=bufs) as negp,
        ):
            import contextlib

            q = io.tile([BS_LOC, D], f32)
            p = io.tile([BS_LOC, D], f32)
            if merged_out:
                # one tile: cols [0:33] DVE (s_pos, s_neg), [33:67] ACT
                # (qq, pp, nn)
                stats = io.tile([BS_LOC, 3 + 2 * NEG], f32)
                dve_stats = stats[:, 0:1 + NEG]
                act_stats = stats[:, 1 + NEG:]
            else:
                # column 0: s_pos, columns 1..32: s_neg   (written by DVE only)
                dve_stats = io.tile([BS_LOC, 1 + NEG], f32)
                # columns 0,1: qq, pp, cols 2..33: nn  (written by ACT only)
                act_stats = io.tile([BS_LOC, 2 + NEG], f32)
            # throwaway full-size outputs of the fused ops (stride-0 write)
            dummy_v = io.tile([BS_LOC, 1], f32)
            dummy_a = io.tile([BS_LOC, 1], f32)
            COPY = mybir.ActivationFunctionType.Copy
            # q/p can ride the ACT HWDGE ring so the first negative chunk
            # starts concurrently on the SP ring
            qp_dma = nc.scalar if qp_engine == "scalar" else nc.sync

            loop_cm = (
                tc.For_i(0, bench_iters, 1) if bench_iters
                else contextlib.nullcontext()
            )
            with loop_cm:
                qp_dma.dma_start(out=q[:], in_=q_ext[:])
                qp_dma.dma_start(out=p[:], in_=p_ext[:])

                # accum_out = sum((q * 1.0) * p) = q.p  (fused DVE dot;
                # standard TensorScalarPtr op: 1.26 us/op on HW vs 1.60 us
                # for the custom affine_mul_reduce)
                nc.vector.scalar_tensor_tensor(
                    out=dummy_v.broadcast_to((BS_LOC, D)),
                    in0=q[:],
                    scalar=1.0,
                    in1=p[:],
                    op0=mybir.AluOpType.mult,
                    op1=mybir.AluOpType.mult,
                    accum_out=dve_stats[:, 0:1],
                )
                nc.scalar.activation(
                    out=dummy_a.broadcast_to((BS_LOC, D)),
                    in_=q[:],
                    func=SQUARE,
                    accum_out=act_stats[:, 0:1],
                )
                nc.scalar.activation(
                    out=dummy_a.broadcast_to((BS_LOC, D)),
                    in_=p[:],
                    func=SQUARE,
                    accum_out=act_stats[:, 1:2],
                )

                j0 = 0
                for ci, ch in enumerate(chunks):
                    negs = negp.tile([BS_LOC, ch * D], f32)
                    ring = nc.scalar if (alternate_rings and ci % 2) else nc.sync
                    ring.dma_start(
                        out=negs[:], in_=n_ext[:, j0 * D:(j0 + ch) * D]
                    )
                    for jj in range(ch):
                        j = j0 + jj
                        sl = negs[:, jj * D:(jj + 1) * D]
                        if gp_every and j % gp_every == gp_every - 1:
                            # hedge: offload this dot to GPSIMD (multiply) +
                            # ACT (Copy-with-accum reduce); both have slack
                            gp_scratch = negp.tile(
                                [BS_LOC, D], f32, tag="gp_scratch"
                            )
                            nc.gpsimd.tensor_mul(
                                out=gp_scratch[:], in0=q[:], in1=sl
                            )
                            nc.scalar.activation(
                                out=dummy_a.broadcast_to((BS_LOC, D)),
                                in_=gp_scratch[:],
                                func=COPY,
                                accum_out=dve_stats[:, 1 + j:2 + j],
                            )
                        else:
                            nc.vector.scalar_tensor_tensor(
                                out=dummy_v.broadcast_to((BS_LOC, D)),
                                in0=q[:],
                                scalar=1.0,
                                in1=sl,
                                op0=mybir.AluOpType.mult,
                                op1=mybir.AluOpType.mult,
                                accum_out=dve_stats[:, 1 + j:2 + j],
                            )
                        nc.scalar.activation(
                            out=dummy_a.broadcast_to((BS_LOC, D)),
                            in_=sl,
                            func=SQUARE,
                            accum_out=act_stats[:, 2 + j:3 + j],
                        )
                    j0 += ch

                if merged_out:
                    nc.sync.dma_start(out=stats_out[:], in_=stats[:])
                else:
                    nc.sync.dma_start(out=dve_out[:], in_=dve_stats[:])
                    nc.sync.dma_start(out=act_out[:], in_=act_stats[:])
    nc.finalize()  # Bacc: runs wait-splitting + register allocation passes
    return nc


def kernel(text_embeddings, text_pos_embeddings, text_neg_embeddings):
    global LAST_RESULT
    from concourse.bass_utils import run_bass_kernel_spmd

    if "nc" not in _CACHE:
        _CACHE["nc"] = _build()
    nc = _CACHE["nc"]

    q = np.ascontiguousarray(np.asarray(text_embeddings, dtype=np.float32))
    p = np.ascontiguousarray(np.asarray(text_pos_embeddings, dtype=np.float32))
    n = np.ascontiguousarray(np.asarray(text_neg_embeddings, dtype=np.float32))

    in_maps = []
    for c in range(N_CORES):
        s0, s1 = c * BS_LOC, (c + 1) * BS_LOC
        in_maps.append(
            {
                "q": q[s0:s1],
                "p": p[s0:s1],
                "n": np.ascontiguousarray(
                    n[s0 * NEG:s1 * NEG].reshape(BS_LOC, NEG * D)
                ),
            }
        )

    res = run_bass_kernel_spmd(nc, in_maps, core_ids=list(range(N_CORES)))
    LAST_RESULT = res

    if "stats" in res.results[0]:
        stats = np.concatenate([r["stats"] for r in res.results], axis=0)
        dve = stats[:, 0:1 + NEG]
        act = stats[:, 1 + NEG:]
    else:
        dve = np.concatenate([r["dve_stats"] for r in res.results], axis=0)
        act = np.concatenate([r["act_stats"] for r in res.results], axis=0)

    s_pos = dve[:, 0].astype(np.float64)
    s_neg = dve[:, 1:].astype(np.float64)
    qq = act[:, 0].astype(np.float64)
    pp = act[:, 1].astype(np.float64)
    nn = act[:, 2:].astype(np.float64)

    q_norm = np.maximum(np.sqrt(qq), EPS)
    p_norm = np.maximum(np.sqrt(pp), EPS)
    n_norm = np.maximum(np.sqrt(nn), EPS)

    sim_pos = s_pos / (q_norm * p_norm)
    sim_neg = s_neg / (q_norm[:, None] * n_norm)
    sim = np.concatenate([sim_pos[:, None], sim_neg], axis=1) / TEMPERATURE

    m = sim.max(axis=1)
    lse = m + np.log(np.exp(sim - m[:, None]).sum(axis=1))
    loss = -(sim[:, 0] - lse).mean() * CLS_W
    return np.asarray(loss, dtype=np.float32)
